# revision 11
# baseline (speedup 1.0000x reference)
"""Trainium2 Bass kernel for nn_CLLayer (SimCLR-style contrastive loss).

Math (reference, tau=0.5):
    h1 = elu(z1 @ W1.T + b1) @ W2.T + b2 ; h2 likewise
    n1, n2 = row-normalized h1, h2
    l1_i = log(sum_j exp(2*n1_i.n1_j) + sum_j exp(2*n1_i.n2_j) - e^2) - 2*n1_i.n2_i
    l2_i = log(sum_j exp(2*n2_i.n2_j) + sum_j exp(2*n1_j.n2_i) - e^2) - 2*n1_i.n2_i
    out = 0.5*(l1+l2)

Strategy (row-parallel over N=8192, 1024 rows/core, 8 cores):
 - FP8(e4m3) DoubleRow matmuls everywhere (projection + similarity): K=256
   per instruction -> 2x bf16 MAC rate on TRN2.
 - S11/S22 are symmetric: each core computes, for EVERY absolute strip j
   (uniform SPMD addresses), only quadrant Q01 (rows 0:512 x cols 512:1024)
   at full weight plus the two diagonal quadrants Q00/Q11 scaled by 1/2
   (exp bias = ln 1/2). The transposed half of each pair-block arrives as
   column sums from the partner core via ReduceScatter. S12 is not
   symmetric -> full strips; its column sums give l2's "between" sums.
 - exp tiles are written in fp8; column sums are ones-vector DoubleRow
   matmuls accumulated in PSUM. Row sums ride on activation accum_out.
 - All DRAM<->SBUF layouts are [128, KO, x] (contiguous) including the
   AllGather payloads; the host pre-arranges inputs into this layout.
 - Sym strips are processed in pairs sharing the stationary lhs tile
   between consecutive matmuls (LdWeights reuse).
 - Normalized embeddings are scaled x16 and cast fp8 before a fp8
   AllGather; sim psums are descaled in the exp (scale=2/256).
 - pos diag p_i = n1_i.n2_i comes from a separate bf16 path (h1*h2 ones
   reduction times f32 1/norms) for accuracy.

Host-side prep: K-major [ki, ko, x] arrangement, fp8 casts, and
b2' = b2 - sum_k W2_f8 so ELU is computed as relu(x) + min(exp(x),1)
(device ELU' = elu + 1).
"""

import math
import os
from functools import lru_cache

import ml_dtypes
import numpy as np

import concourse.bacc as bacc
import concourse.bass as bass
import concourse.mybir as mybir
import concourse.tile as tile
from concourse.bass_utils import run_bass_kernel_spmd

N, D = 8192, 1024
NCORES = 8
BLK = N // NCORES  # 1024
P = 128
KO = D // P  # 8 k-tiles
KO2 = KO // 2  # 4 double-row k-pairs
NT = BLK // P  # 8 i-tiles per core
E2 = float(np.exp(2.0))  # exp(1/tau), tau=0.5
SC = 2.0 / 256.0  # exp scale: tau and the 16x16 fp8 prescale
LN_HALF = float(math.log(0.5))
BF = mybir.dt.bfloat16
F32 = mybir.dt.float32
F8 = mybir.dt.float8e4
AF = mybir.ActivationFunctionType
ALU = mybir.AluOpType
DR = mybir.MatmulPerfMode.DoubleRow


def _build():
    nc = bacc.Bacc("TRN2", target_bir_lowering=False, debug=False, num_devices=NCORES)

    # all tensors arrive pre-arranged as [ki=128, ko, x] (contiguous loads)
    z1t = nc.dram_tensor("z1t", [P, KO, BLK], F8, kind="ExternalInput")
    z2t = nc.dram_tensor("z2t", [P, KO, BLK], F8, kind="ExternalInput")
    w1t = nc.dram_tensor("w1t", [P, KO, D], F8, kind="ExternalInput")
    w2t = nc.dram_tensor("w2t", [P, KO, D], F8, kind="ExternalInput")
    b1 = nc.dram_tensor("b1", [D], F32, kind="ExternalInput")
    b2p = nc.dram_tensor("b2p", [D], F32, kind="ExternalInput")
    out = nc.dram_tensor("out", [BLK], F32, kind="ExternalOutput")

    pt = lambda ap: ap.rearrange("(t p) -> p t", p=P)  # [1024] -> [128, 8]

    with tile.TileContext(nc) as tc:
        with (
            tc.tile_pool(name="consts", bufs=1) as consts,
            tc.tile_pool(name="mats", bufs=1) as mats,
            tc.tile_pool(name="strip", bufs=1) as strip,
            tc.tile_pool(name="scratch", bufs=2) as scratch,
            tc.tile_pool(name="rhs", bufs=4) as rhsp,
            tc.tile_pool(name="expp", bufs=6) as expp,
            tc.tile_pool(name="small", bufs=1) as small,
            tc.tile_pool(name="psA", bufs=3, space="PSUM") as psA,
            tc.tile_pool(name="psB", bufs=2, space="PSUM") as psB,
            tc.tile_pool(name="dram", bufs=1, space="DRAM") as dram,
        ):
            # ---------------- constants ----------------
            w1_sb = consts.tile([P, KO, D], F8)
            w2_sb = consts.tile([P, KO, D], F8)
            b1_sb = consts.tile([P, KO], F32)
            b2_sb = consts.tile([P, KO], F32)
            z1_sb = mats.tile([P, KO, BLK], F8, tag="z1")
            z2_sb = mats.tile([P, KO, BLK], F8, tag="z2")
            # k-chunked so the first matmuls can start after the first chunk
            for k2 in range(KO2):
                ksl = slice(2 * k2, 2 * k2 + 2)
                nc.sync.dma_start(w1_sb[:, ksl, :], w1t[:, ksl, :])
                nc.sync.dma_start(z1_sb[:, ksl, :], z1t[:, ksl, :])
            nc.sync.dma_start(w2_sb[:], w2t[:])
            nc.sync.dma_start(z2_sb[:], z2t[:])
            nc.sync.dma_start(b1_sb[:], pt(b1[:]))
            nc.sync.dma_start(b2_sb[:], pt(b2p[:]))
            ones8 = consts.tile([P, 2, 16], F8)
            ones_bf = consts.tile([P, 1], BF)
            lnhalf = consts.tile([P, 1], F32)
            negE2 = consts.tile([P, 1], F32)
            nc.vector.memset(ones8[:], 1.0)
            nc.vector.memset(ones_bf[:], 1.0)
            nc.vector.memset(lnhalf[:], LN_HALF)
            nc.vector.memset(negE2[:], -E2)

            h1_sb = mats.tile([P, KO, BLK], BF, tag="h1")
            h2_sb = mats.tile([P, KO, BLK], BF, tag="h2")
            n1_f8 = mats.tile([P, KO, BLK], F8, tag="n1")
            n2_f8 = mats.tile([P, KO, BLK], F8, tag="n2")

            ag1_in = dram.tile([P, KO, BLK], F8)
            ag2_in = dram.tile([P, KO, BLK], F8)
            ag1_out = dram.tile([NCORES, P, KO, BLK], F8, addr_space="Shared")
            ag2_out = dram.tile([NCORES, P, KO, BLK], F8, addr_space="Shared")
            rs_in = dram.tile([NCORES, 3, BLK], F32)
            rs_out = dram.tile([3, BLK], F32)
            rn_dram = dram.tile([2, BLK], BF)
            p_dram = dram.tile([BLK], F32)

            # rowsum partials: one column per (strip, quadrant-group)
            D1p = strip.tile([P, NT, 3 * NCORES], F32)  # S11: 2/strip, S12: 1/strip
            D2p = strip.tile([P, NT, 2 * NCORES], F32)  # S22: 2/strip
            nc.vector.memset(D1p[:], 0.0)
            nc.vector.memset(D2p[:], 0.0)

            rn_f = [
                small.tile([1, BLK], F32, tag=f"rn_f{i}", name=f"rn_f{i}")
                for i in range(2)
            ]

            def dr_multi(ps_list, lhs, tt, rhs_list):
                """K=1024 fp8 DoubleRow accumulation over several (ps, rhs)
                streams sharing the same stationary lhs tile per k-pair."""
                for k2 in range(KO2):
                    lslice = lhs[:, 2 * k2 : 2 * k2 + 2, bass.ts(tt, P)]
                    for ps_sl, (rt, col) in zip(ps_list, rhs_list):
                        nc.tensor.matmul(
                            ps_sl,
                            lslice,
                            rt[:, 2 * k2 : 2 * k2 + 2, bass.ds(col, 512)],
                            start=(k2 == 0),
                            stop=(k2 == KO2 - 1),
                            perf_mode=DR,
                        )

            # ------------ projection + normalize, per tensor ------------
            def proj_layer(w_sb, src, emit_ot, ots=None):
                for ot in ots if ots is not None else range(KO):
                    ps = psA.tile([P, 1024], F32, tag="ps_big")
                    dr_multi(
                        [ps[:, 0:512], ps[:, 512:1024]],
                        w_sb, ot, [(src, 0), (src, 512)],
                    )
                    emit_ot(ot, ps)

            def l1_emit(elu_sb):
                def emit(ot, ps):
                    bcol = b1_sb[:, ot : ot + 1]
                    e_t = scratch.tile([P, 1024], F32, tag="e_t")
                    r_t = scratch.tile([P, 1024], F32, tag="r_t")
                    nc.scalar.activation(e_t[:], ps[:], AF.Exp, bias=bcol)
                    nc.scalar.activation(r_t[:], ps[:], AF.Relu, bias=bcol)
                    nc.vector.tensor_scalar(e_t[:], e_t[:], 1.0, None, ALU.min)
                    nc.vector.tensor_tensor(elu_sb[:, ot, :], e_t[:], r_t[:], ALU.add)
                return emit

            def l2_emit(h_sb):
                def emit(ot, ps):
                    nc.vector.tensor_scalar(
                        h_sb[:, ot, :], ps[:], b2_sb[:, ot : ot + 1], None, ALU.add
                    )
                return emit

            def normalize(h_sb, n_f8, rn_slot):
                # sumsq over d via bf16 ones-matmul on h*h
                ssps = [
                    psB.tile([16, 512], F32, name=f"ssps{rn_slot}_{c}", tag="ps_wide")
                    for c in range(2)
                ]
                for kt in range(KO):
                    sq = scratch.tile([P, BLK], BF, tag="sq")
                    nc.vector.tensor_tensor(sq[:], h_sb[:, kt, :], h_sb[:, kt, :], ALU.mult)
                    for ch in range(2):
                        nc.tensor.matmul(
                            ssps[ch][0:1, :],
                            ones_bf[:],
                            sq[:, bass.ts(ch, 512)],
                            start=(kt == 0),
                            stop=(kt == KO - 1),
                        )
                # rn = 1/||h||: fast reciprocal + one Newton step
                rn16_bf = small.tile([1, BLK], BF, tag="rn16_bf", name=f"rn16_{rn_slot}")
                for ch in range(2):
                    sl = bass.ts(ch, 512)
                    ssq_c = small.tile([1, 512], F32, tag="ssq_c", name=f"ssq{rn_slot}{ch}")
                    nrm_c = small.tile([1, 512], F32, tag="nrm_c", name=f"nrm{rn_slot}{ch}")
                    y_c = small.tile([1, 512], F32, tag="y_c", name=f"y{rn_slot}{ch}")
                    t1_c = small.tile([1, 512], F32, tag="t1_c", name=f"t1{rn_slot}{ch}")
                    nc.vector.tensor_copy(ssq_c[:], ssps[ch][0:1, :])
                    nc.scalar.activation(nrm_c[:], ssps[ch][0:1, :], AF.Sqrt)
                    nc.vector.reciprocal_approx_fast(y_c[:], nrm_c[:])
                    nc.vector.tensor_tensor(t1_c[:], y_c[:], y_c[:], ALU.mult)
                    nc.vector.tensor_tensor(t1_c[:], t1_c[:], ssq_c[:], ALU.mult)
                    nc.vector.tensor_scalar(t1_c[:], t1_c[:], -0.5, 1.5, ALU.mult, ALU.add)
                    nc.vector.tensor_tensor(t1_c[:], y_c[:], t1_c[:], ALU.mult)
                    nc.vector.tensor_copy(rn_f[rn_slot][:, sl], t1_c[:])
                    nc.vector.tensor_scalar(t1_c[:], t1_c[:], 16.0, None, ALU.mult)
                    nc.vector.tensor_copy(rn16_bf[:, sl], t1_c[:])
                nc.scalar.dma_start(rn_dram[rn_slot : rn_slot + 1, :], rn16_bf[:])
                rn_bc = scratch.tile([P, BLK], BF, tag="rnbc", bufs=2, name=f"rnbc{rn_slot}")
                nc.scalar.dma_start(
                    rn_bc[:], rn_dram[rn_slot : rn_slot + 1, :].to_broadcast((P, BLK))
                )
                for kt in range(KO):
                    nc.vector.tensor_tensor(
                        n_f8[:, kt, :], h_sb[:, kt, :], rn_bc[:], ALU.mult
                    )

            rg = [list(range(NCORES))]
            elu1 = mats.tile([P, KO, BLK], F8, tag="elu", name="elu1")
            elu2 = mats.tile([P, KO, BLK], F8, tag="z1", name="elu2")  # z1 dead post-l1

            # interleave the two projections to fill PE pipeline bubbles while
            # keeping proj1 (the AG1 critical path) in front
            l1e2 = l1_emit(elu2)
            proj_layer(w1_sb, z1_sb, l1_emit(elu1))
            proj_layer(w1_sb, z2_sb, l1e2, ots=range(2))  # covers the elu1 drain
            proj_layer(w2_sb, elu1, l2_emit(h1_sb))
            normalize(h1_sb, n1_f8, 0)
            nc.scalar.dma_start(ag1_in[:], n1_f8[:])
            nc.gpsimd.collective_compute(
                "AllGather", ALU.bypass, replica_groups=rg,
                ins=[ag1_in[:].opt()], outs=[ag1_out[:].opt()],
            )
            proj_layer(w1_sb, z2_sb, l1e2, ots=range(2, KO))
            proj_layer(w2_sb, elu2, l2_emit(h2_sb))
            normalize(h2_sb, n2_f8, 1)
            nc.scalar.dma_start(ag2_in[:], n2_f8[:])
            nc.gpsimd.collective_compute(
                "AllGather", ALU.bypass, replica_groups=rg,
                ins=[ag2_in[:].opt()], outs=[ag2_out[:].opt()],
            )

            # ------ p_i = n1_i . n2_i via bf16 h1*h2 and f32 1/norms ------
            pps = [
                psB.tile([16, 512], F32, name=f"pps{c}", tag="ps_wide") for c in range(2)
            ]
            for kt in range(KO):
                hq = scratch.tile([P, BLK], BF, tag="sq", name=f"hq{kt}")
                nc.vector.tensor_tensor(hq[:], h1_sb[:, kt, :], h2_sb[:, kt, :], ALU.mult)
                for ch in range(2):
                    nc.tensor.matmul(
                        pps[ch][0:1, :],
                        ones_bf[:],
                        hq[:, bass.ts(ch, 512)],
                        start=(kt == 0),
                        stop=(kt == KO - 1),
                    )
            for ch in range(2):
                sl = bass.ts(ch, 512)
                p_c = small.tile([1, 512], F32, tag="ssq_c", name=f"p_c{ch}")
                nc.vector.tensor_copy(p_c[:], pps[ch][0:1, :])
                nc.vector.tensor_tensor(p_c[:], p_c[:], rn_f[0][:, sl], ALU.mult)
                nc.vector.tensor_tensor(p_c[:], p_c[:], rn_f[1][:, sl], ALU.mult)
                nc.gpsimd.dma_start(p_dram[ch * 512 : (ch + 1) * 512], p_c[:])

            # ---------------- similarity passes ----------------
            def rhs_load(ag, j, nm):
                t = rhsp.tile([P, KO, BLK], F8, tag="rhs", name=nm)
                blk = ag[j]
                nc.sync.dma_start(t[:, 0:4, :], blk[:, 0:4, :])
                nc.sync.dma_start(t[:, 4:8, :], blk[:, 4:8, :])
                return t

            # colsums deferred to the next strip-group so the PE never waits
            # on the Act engine's exp outputs
            pending = []

            def flush_pending():
                while pending:
                    pending.pop(0)()

            def colsum(j, rs_slot, h, ex_tiles, nm):
                """PSUM-accumulated fp8 ones DoubleRow colsum -> rs_in[j, slot, h]."""

                def emit():
                    cps = psB.tile([16, 512], F32, tag="ps_wide", name=f"cps{nm}")
                    for i, (ext, esl) in enumerate(ex_tiles):
                        nc.tensor.matmul(
                            cps[:],
                            ones8[:],
                            ext[:, :, esl] if esl is not None else ext[:],
                            start=(i == 0),
                            stop=(i == len(ex_tiles) - 1),
                            perf_mode=DR,
                        )
                    cst = scratch.tile([1, 512], F32, tag="cst", bufs=2, name=f"cst{nm}")
                    nc.vector.tensor_copy(cst[:], cps[0:1, :])
                    nc.gpsimd.dma_start(
                        rs_in[j : j + 1, rs_slot : rs_slot + 1, bass.ds(h * 512, 512)],
                        cst[:],
                    )

                pending.append(emit)

            def sym_pair(lhs, ag, j0, Dp, rs_slot, nm):
                """Two symmetric-half strips (j0, j0+1), lhs shared per matmul
                pair: Q01 full + Q00/Q11 at exp weight 1/2."""
                rt0 = rhs_load(ag, j0, f"r{nm}a")
                rt1 = rhs_load(ag, j0 + 1, f"r{nm}b")
                ex = {}  # (strip, quad, tt-pair) -> fp8 exp tile
                first = True
                # Q00 tts 0-3 cols 0:512 w=1/2; Q01 tts 0-3 cols 512: w=1;
                # Q11 tts 4-7 cols 512: w=1/2
                for quad, tts, col, half in (
                    ("q00", range(4), 0, True),
                    ("q01", range(4), 512, False),
                    ("q11", range(4, 8), 512, True),
                ):
                    for tt in tts:
                        ps = psA.tile([P, 1024], F32, tag="ps_big", name=f"p{nm}{quad}{tt}")
                        dr_multi(
                            [ps[:, 0:512], ps[:, 512:1024]],
                            lhs, tt, [(rt0, col), (rt1, col)],
                        )
                        if first:
                            flush_pending()
                            first = False
                        for s in range(2):
                            slot = 2 * (j0 + s) + (0 if quad == "q01" else 1)
                            key = (s, quad, tt // 2)
                            if key not in ex:
                                ex[key] = expp.tile(
                                    [P, 2, 512], F8, tag="exq", bufs=24,
                                    name=f"x{nm}{s}{quad}{tt // 2}",
                                )
                            nc.scalar.activation(
                                ex[key][:, tt % 2, :], ps[:, bass.ts(s, 512)], AF.Exp,
                                scale=SC, bias=(lnhalf[:] if half else 0.0),
                                accum_out=Dp[:, tt, slot : slot + 1],
                            )
                for s in range(2):
                    colsum(
                        j0 + s, rs_slot, 0,
                        [(ex[(s, "q00", 0)], None), (ex[(s, "q00", 1)], None)],
                        f"a{nm}{s}",
                    )
                    colsum(
                        j0 + s, rs_slot, 1,
                        [(ex[(s, "q01", 0)], None), (ex[(s, "q01", 1)], None),
                         (ex[(s, "q11", 2)], None), (ex[(s, "q11", 3)], None)],
                        f"b{nm}{s}",
                    )

            # S11 (needs AG1 only; overlaps AG2)
            for j0 in range(0, NCORES, 2):
                sym_pair(n1_f8, ag1_out, j0, D1p, 0, f"s11_{j0}")
            # S22 (needs AG2)
            for j0 in range(0, NCORES, 2):
                sym_pair(n2_f8, ag2_out, j0, D2p, 1, f"s22_{j0}")
            # S12 full strips (lhs n1, rhs gathered n2)
            for j in range(NCORES):
                rt = rhs_load(ag2_out, j, f"s12_{j}")
                exf = [
                    expp.tile([P, 2, 1024], F8, tag="exf", bufs=8, name=f"xf{j}_{i}")
                    for i in range(4)
                ]
                for tt in range(NT):
                    ps = psA.tile([P, 1024], F32, tag="ps_big", name=f"pf{j}_{tt}")
                    dr_multi(
                        [ps[:, 0:512], ps[:, 512:1024]],
                        n1_f8, tt, [(rt, 0), (rt, 512)],
                    )
                    if tt == 0:
                        flush_pending()
                    nc.scalar.activation(
                        exf[tt // 2][:, tt % 2, :], ps[:], AF.Exp, scale=SC,
                        accum_out=D1p[:, tt, 2 * NCORES + j : 2 * NCORES + j + 1],
                    )
                for h in range(2):
                    colsum(
                        j, 2, h,
                        [(t, bass.ds(h * 512, 512)) for t in exf],
                        f"f{j}_{h}",
                    )
            flush_pending()

            nc.gpsimd.collective_compute(
                "ReduceScatter", ALU.add, replica_groups=rg,
                ins=[rs_in[:].opt()], outs=[rs_out[:].opt()],
            )

            # ---------------- final loss ----------------
            # pm = -2p is ready long before the sims end
            p2 = small.tile([P, NT], F32, tag="p2")
            nc.sync.dma_start(p2[:], pt(p_dram[:]))
            pm = small.tile([P, NT], F32, tag="pm")
            nc.vector.tensor_scalar(pm[:], p2[:], -2.0, None, ALU.mult)

            r1s = small.tile([P, NT], F32, tag="r1s")
            r2s = small.tile([P, NT], F32, tag="r2s")
            nc.vector.reduce_sum(r1s[:], D1p[:], axis=mybir.AxisListType.X)
            nc.vector.reduce_sum(r2s[:], D2p[:], axis=mybir.AxisListType.X)
            c1 = small.tile([P, NT], F32, tag="c1")
            c2a = small.tile([P, NT], F32, tag="c2a")
            c2b = small.tile([P, NT], F32, tag="c2b")
            nc.sync.dma_start(c1[:], pt(rs_out[0]))
            nc.sync.dma_start(c2a[:], pt(rs_out[1]))
            nc.sync.dma_start(c2b[:], pt(rs_out[2]))

            d1 = small.tile([P, NT], F32, tag="d1")
            d2 = small.tile([P, NT], F32, tag="d2")
            nc.vector.tensor_tensor(d1[:], r1s[:], c1[:], ALU.add)
            nc.vector.tensor_tensor(d2[:], r2s[:], c2a[:], ALU.add)
            nc.vector.tensor_tensor(d2[:], d2[:], c2b[:], ALU.add)
            l1 = small.tile([P, NT], F32, tag="l1")
            l2 = small.tile([P, NT], F32, tag="l2")
            nc.scalar.activation(l1[:], d1[:], AF.Ln, bias=negE2[:])
            nc.scalar.activation(l2[:], d2[:], AF.Ln, bias=negE2[:])
            loss = small.tile([P, NT], F32, tag="loss")
            nc.vector.tensor_tensor(loss[:], l1[:], l2[:], ALU.add)
            nc.vector.tensor_scalar(loss[:], loss[:], 0.5, None, ALU.mult)
            nc.vector.tensor_tensor(loss[:], loss[:], pm[:], ALU.add)
            nc.sync.dma_start(pt(out[:]), loss[:])

    nc.finalize()
    return nc


@lru_cache(maxsize=1)
def _built():
    return _build()


def _kimajor(a):
    """[D, X] -> [ki=128, ko=8, X] contiguous device layout."""
    d, x = a.shape
    return np.ascontiguousarray(a.reshape(KO, P, x).transpose(1, 0, 2))


def _prep_inputs(z1, z2, fc1_w, fc1_b, fc2_w, fc2_b):
    f8 = ml_dtypes.float8_e4m3
    w1 = np.asarray(fc1_w, np.float32)
    w2 = np.asarray(fc2_w, np.float32)
    w1t = _kimajor(np.ascontiguousarray(w1.T).astype(f8))
    w2t = _kimajor(np.ascontiguousarray(w2.T).astype(f8))
    b1 = np.asarray(fc1_b, np.float32)
    # device computes (elu+1) @ W2.T; correct with the fp8-rounded W2 row sums
    b2p = (
        np.asarray(fc2_b, np.float32) - w2.astype(f8).astype(np.float32).sum(axis=1)
    ).astype(np.float32)
    in_maps = []
    for c in range(NCORES):
        sl = slice(c * BLK, (c + 1) * BLK)
        in_maps.append(
            {
                "z1t": _kimajor(np.asarray(z1[sl], np.float32).T.astype(f8)),
                "z2t": _kimajor(np.asarray(z2[sl], np.float32).T.astype(f8)),
                "w1t": w1t,
                "w2t": w2t,
                "b1": b1,
                "b2p": b2p,
            }
        )
    return in_maps


def _install_ntff_shim():
    """Register the axon NTFF profile hook (antenv.axon_hooks is absent in
    this image; rebuild it from trn_agent_boot's ctypes recipe)."""
    import sys
    import types

    if "antenv.axon_hooks" in sys.modules:
        return True
    try:
        import antenv
        from trn_agent_boot.trn_boot import _ntff_profile_via_ctypes

        hook = _ntff_profile_via_ctypes("/opt/axon/libaxon_pjrt.so")
        if hook is None:
            return False
        m = types.ModuleType("antenv.axon_hooks")
        m._hook = hook
        m.get_axon_ntff_profile_hook = lambda: m._hook
        m.set_axon_ntff_profile_hook = lambda h: setattr(m, "_hook", h)
        sys.modules["antenv.axon_hooks"] = m
        antenv.axon_hooks = m
        # artifact upload needs egress; neuter it for local profiling
        import concourse.bass_utils as _bu

        _bu.upload_artifacts = lambda tmpdir: f"file://{tmpdir}"
        return True
    except Exception as e:
        print(f"ntff shim unavailable: {e!r}")
        return False


def _run(in_maps, trace=False):
    nc = _built()
    if trace and not _install_ntff_shim():
        trace = False
    last = None
    for attempt in range(3):
        try:
            res = run_bass_kernel_spmd(nc, in_maps, list(range(NCORES)), trace=trace)
            if all(np.isfinite(res.results[c]["out"]).all() for c in range(NCORES)):
                return res
            print("nonfinite output, retrying")
        except Exception as e:  # device occasionally wedged from a prior process
            last = e
            if "UNRECOVERABLE" not in str(e) and "UNAVAILABLE" not in str(e):
                raise
            print(f"device error (attempt {attempt}): retrying")
    if last is not None:
        raise last
    return res


def kernel(z1, z2, fc1_w, fc1_b, fc2_w, fc2_b):
    in_maps = _prep_inputs(z1, z2, fc1_w, fc1_b, fc2_w, fc2_b)
    res = _run(in_maps, trace=os.environ.get("KERNEL_TRACE", "") == "1")
    if res.exec_time_ns is not None:
        print(f"HW exec time: {res.exec_time_ns} ns")
    out = np.concatenate([res.results[c]["out"] for c in range(NCORES)])
    return out.astype(np.float32)


# revision 13
# speedup vs baseline: 1.0312x; 1.0312x over previous
"""Trainium2 Bass kernel for nn_CLLayer (SimCLR-style contrastive loss).

Math (reference, tau=0.5):
    h1 = elu(z1 @ W1.T + b1) @ W2.T + b2 ; h2 likewise
    n1, n2 = row-normalized h1, h2
    l1_i = log(sum_j exp(2*n1_i.n1_j) + sum_j exp(2*n1_i.n2_j) - e^2) - 2*n1_i.n2_i
    l2_i = log(sum_j exp(2*n2_i.n2_j) + sum_j exp(2*n1_j.n2_i) - e^2) - 2*n1_i.n2_i
    out = 0.5*(l1+l2)

Strategy (row-parallel over N=8192, 1024 rows/core, 8 cores):
 - FP8(e4m3) DoubleRow matmuls everywhere (projection + similarity): K=256
   per instruction -> 2x bf16 MAC rate on TRN2.
 - S11/S22 are symmetric: each core computes, for EVERY absolute strip j
   (uniform SPMD addresses), only quadrant Q01 (rows 0:512 x cols 512:1024)
   at full weight plus the two diagonal quadrants Q00/Q11 scaled by 1/2
   (exp bias = ln 1/2). The transposed half of each pair-block arrives as
   column sums from the partner core via ReduceScatter. S12 is not
   symmetric -> full strips; its column sums give l2's "between" sums.
 - exp tiles are written in fp8; column sums are ones-vector DoubleRow
   matmuls accumulated in PSUM. Row sums ride on activation accum_out.
 - All DRAM<->SBUF layouts are [128, KO, x] (contiguous) including the
   AllGather payloads; the host pre-arranges inputs into this layout.
 - Sym strips are processed in pairs sharing the stationary lhs tile
   between consecutive matmuls (LdWeights reuse).
 - Normalized embeddings are scaled x16 and cast fp8 before a fp8
   AllGather; sim psums are descaled in the exp (scale=2/256).
 - pos diag p_i = n1_i.n2_i comes from a separate bf16 path (h1*h2 ones
   reduction times f32 1/norms) for accuracy.

Host-side prep: K-major [ki, ko, x] arrangement, fp8 casts, and
b2' = b2 - sum_k W2_f8 so ELU is computed as relu(x) + min(exp(x),1)
(device ELU' = elu + 1).
"""

import math
import os
from functools import lru_cache

import ml_dtypes
import numpy as np

import concourse.bacc as bacc
import concourse.bass as bass
import concourse.mybir as mybir
import concourse.tile as tile
from concourse.bass_utils import run_bass_kernel_spmd

N, D = 8192, 1024
NCORES = 8
BLK = N // NCORES  # 1024
P = 128
KO = D // P  # 8 k-tiles
KO2 = KO // 2  # 4 double-row k-pairs
NT = BLK // P  # 8 i-tiles per core
E2 = float(np.exp(2.0))  # exp(1/tau), tau=0.5
SC = 2.0 / 256.0  # exp scale: tau and the 16x16 fp8 prescale
LN_HALF = float(math.log(0.5))
BF = mybir.dt.bfloat16
F32 = mybir.dt.float32
F8 = mybir.dt.float8e4
AF = mybir.ActivationFunctionType
ALU = mybir.AluOpType
DR = mybir.MatmulPerfMode.DoubleRow


def _build():
    nc = bacc.Bacc("TRN2", target_bir_lowering=False, debug=False, num_devices=NCORES)

    # all tensors arrive pre-arranged as [ki=128, ko, x] (contiguous loads)
    z1t = nc.dram_tensor("z1t", [P, KO, BLK], F8, kind="ExternalInput")
    z2t = nc.dram_tensor("z2t", [P, KO, BLK], F8, kind="ExternalInput")
    w1t = nc.dram_tensor("w1t", [P, KO, D], F8, kind="ExternalInput")
    w2t = nc.dram_tensor("w2t", [P, KO, D], F8, kind="ExternalInput")
    b1 = nc.dram_tensor("b1", [D], F32, kind="ExternalInput")
    b2p = nc.dram_tensor("b2p", [D], F32, kind="ExternalInput")
    out = nc.dram_tensor("out", [BLK], F32, kind="ExternalOutput")

    pt = lambda ap: ap.rearrange("(t p) -> p t", p=P)  # [1024] -> [128, 8]

    with tile.TileContext(nc) as tc:
        with (
            tc.tile_pool(name="consts", bufs=1) as consts,
            tc.tile_pool(name="mats", bufs=1) as mats,
            tc.tile_pool(name="strip", bufs=1) as strip,
            tc.tile_pool(name="scratch", bufs=2) as scratch,
            tc.tile_pool(name="rhs", bufs=4) as rhsp,
            tc.tile_pool(name="expp", bufs=6) as expp,
            tc.tile_pool(name="small", bufs=1) as small,
            tc.tile_pool(name="psA", bufs=3, space="PSUM") as psA,
            tc.tile_pool(name="psB", bufs=2, space="PSUM") as psB,
            tc.tile_pool(name="dram", bufs=1, space="DRAM") as dram,
        ):
            # ---------------- constants ----------------
            w1_sb = consts.tile([P, KO, D], F8)
            w2_sb = consts.tile([P, KO, D], F8)
            b1_sb = consts.tile([P, KO], F32)
            b2_sb = consts.tile([P, KO], F32)
            z1_sb = mats.tile([P, KO, BLK], F8, tag="z1")
            z2_sb = mats.tile([P, KO, BLK], F8, tag="z2")
            # k-chunked so the first matmuls can start after the first chunk
            for k2 in range(KO2):
                ksl = slice(2 * k2, 2 * k2 + 2)
                nc.sync.dma_start(w1_sb[:, ksl, :], w1t[:, ksl, :])
                nc.sync.dma_start(z1_sb[:, ksl, :], z1t[:, ksl, :])
            nc.sync.dma_start(w2_sb[:], w2t[:])
            nc.sync.dma_start(z2_sb[:], z2t[:])
            nc.sync.dma_start(b1_sb[:], pt(b1[:]))
            nc.sync.dma_start(b2_sb[:], pt(b2p[:]))
            ones8 = consts.tile([P, 2, 16], F8)
            ones_bf = consts.tile([P, 1], BF)
            lnhalf = consts.tile([P, 1], F32)
            negE2 = consts.tile([P, 1], F32)
            nc.vector.memset(ones8[:], 1.0)
            nc.vector.memset(ones_bf[:], 1.0)
            nc.vector.memset(lnhalf[:], LN_HALF)
            nc.vector.memset(negE2[:], -E2)

            h1_sb = mats.tile([P, KO, BLK], BF, tag="h1")
            h2_sb = mats.tile([P, KO, BLK], BF, tag="h2")
            n1_f8 = mats.tile([P, KO, BLK], F8, tag="n1")
            n2_f8 = mats.tile([P, KO, BLK], F8, tag="n2")

            ag1_in = dram.tile([P, KO, BLK], F8)
            ag2_in = dram.tile([P, KO, BLK], F8)
            ag1_out = dram.tile([NCORES, P, KO, BLK], F8, addr_space="Shared")
            ag2_out = dram.tile([NCORES, P, KO, BLK], F8, addr_space="Shared")
            rs_in_a = dram.tile([NCORES, 2, BLK], F32)  # S11 / S22 colsums
            rs_in_b = dram.tile([NCORES, BLK], F32)  # S12 colsums
            rs_out_a = dram.tile([2, BLK], F32)
            rs_out_b = dram.tile([BLK], F32)
            p_dram = dram.tile([BLK], F32)

            # rowsum partials: one column per (strip, quadrant-group)
            D1p = strip.tile([P, NT, 3 * NCORES], F32)  # S11: 2/strip, S12: 1/strip
            D2p = strip.tile([P, NT, 2 * NCORES], F32)  # S22: 2/strip
            nc.vector.memset(D1p[:], 0.0)
            nc.vector.memset(D2p[:], 0.0)

            rn_f = [
                small.tile([1, BLK], F32, tag=f"rn_f{i}", name=f"rn_f{i}")
                for i in range(2)
            ]

            def dr_multi(ps_list, lhs, tt, rhs_list):
                """K=1024 fp8 DoubleRow accumulation over several (ps, rhs)
                streams sharing the same stationary lhs tile per k-pair."""
                for k2 in range(KO2):
                    lslice = lhs[:, 2 * k2 : 2 * k2 + 2, bass.ts(tt, P)]
                    for ps_sl, (rt, col) in zip(ps_list, rhs_list):
                        nc.tensor.matmul(
                            ps_sl,
                            lslice,
                            rt[:, 2 * k2 : 2 * k2 + 2, bass.ds(col, 512)],
                            start=(k2 == 0),
                            stop=(k2 == KO2 - 1),
                            perf_mode=DR,
                        )

            # ------------ projection + normalize, per tensor ------------
            def proj_layer(w_sb, src, emit_ot, ots=None):
                for ot in ots if ots is not None else range(KO):
                    ps = psA.tile([P, 1024], F32, tag="ps_big")
                    dr_multi(
                        [ps[:, 0:512], ps[:, 512:1024]],
                        w_sb, ot, [(src, 0), (src, 512)],
                    )
                    emit_ot(ot, ps)

            def l1_emit(elu_sb):
                def emit(ot, ps):
                    bcol = b1_sb[:, ot : ot + 1]
                    e_t = scratch.tile([P, 1024], F32, tag="e_t")
                    r_t = scratch.tile([P, 1024], F32, tag="r_t")
                    nc.scalar.activation(e_t[:], ps[:], AF.Exp, bias=bcol)
                    nc.scalar.activation(r_t[:], ps[:], AF.Relu, bias=bcol)
                    nc.vector.tensor_scalar(e_t[:], e_t[:], 1.0, None, ALU.min)
                    nc.vector.tensor_tensor(elu_sb[:, ot, :], e_t[:], r_t[:], ALU.add)
                return emit

            def l2_emit(h_sb):
                def emit(ot, ps):
                    nc.scalar.activation(
                        h_sb[:, ot, :], ps[:], AF.Identity, bias=b2_sb[:, ot : ot + 1]
                    )
                return emit

            def normalize(h_sb, n_f8, rn_slot):
                # sumsq over d via bf16 ones-matmul on h*h
                ssps = [
                    psB.tile([16, 512], F32, name=f"ssps{rn_slot}_{c}", tag="ps_wide")
                    for c in range(2)
                ]
                for kt in range(KO):
                    sq = scratch.tile([P, BLK], BF, tag="sq")
                    nc.vector.tensor_tensor(sq[:], h_sb[:, kt, :], h_sb[:, kt, :], ALU.mult)
                    for ch in range(2):
                        nc.tensor.matmul(
                            ssps[ch][0:1, :],
                            ones_bf[:],
                            sq[:, bass.ts(ch, 512)],
                            start=(kt == 0),
                            stop=(kt == KO - 1),
                        )
                # rn = 1/||h||: fast reciprocal + one Newton step
                rn16_bf = small.tile([1, BLK], BF, tag="rn16_bf", name=f"rn16_{rn_slot}")
                for ch in range(2):
                    sl = bass.ts(ch, 512)
                    ssq_c = small.tile([1, 512], F32, tag="ssq_c", name=f"ssq{rn_slot}{ch}")
                    nrm_c = small.tile([1, 512], F32, tag="nrm_c", name=f"nrm{rn_slot}{ch}")
                    y_c = small.tile([1, 512], F32, tag="y_c", name=f"y{rn_slot}{ch}")
                    t1_c = small.tile([1, 512], F32, tag="t1_c", name=f"t1{rn_slot}{ch}")
                    nc.vector.tensor_copy(ssq_c[:], ssps[ch][0:1, :])
                    nc.scalar.activation(nrm_c[:], ssps[ch][0:1, :], AF.Sqrt)
                    nc.vector.reciprocal_approx_fast(y_c[:], nrm_c[:])
                    nc.vector.tensor_tensor(t1_c[:], y_c[:], y_c[:], ALU.mult)
                    nc.vector.tensor_tensor(t1_c[:], t1_c[:], ssq_c[:], ALU.mult)
                    nc.vector.tensor_scalar(t1_c[:], t1_c[:], -0.5, 1.5, ALU.mult, ALU.add)
                    nc.vector.tensor_tensor(t1_c[:], y_c[:], t1_c[:], ALU.mult)
                    nc.vector.tensor_copy(rn_f[rn_slot][:, sl], t1_c[:])
                    nc.vector.tensor_scalar(t1_c[:], t1_c[:], 16.0, None, ALU.mult)
                    nc.vector.tensor_copy(rn16_bf[:, sl], t1_c[:])
                rn_bc = scratch.tile([P, BLK], BF, tag="rnbc", bufs=2, name=f"rnbc{rn_slot}")
                nc.gpsimd.partition_broadcast(rn_bc[:], rn16_bf[:])
                for kt in range(KO):
                    nc.vector.tensor_tensor(
                        n_f8[:, kt, :], h_sb[:, kt, :], rn_bc[:], ALU.mult
                    )

            rg = [list(range(NCORES))]
            elu1 = mats.tile([P, KO, BLK], F8, tag="elu", name="elu1")
            elu2 = mats.tile([P, KO, BLK], F8, tag="z1", name="elu2")  # z1 dead post-l1

            # interleave the two projections to fill PE pipeline bubbles while
            # keeping proj1 (the AG1 critical path) in front
            l1e2 = l1_emit(elu2)
            proj_layer(w1_sb, z1_sb, l1_emit(elu1))
            proj_layer(w1_sb, z2_sb, l1e2, ots=range(2))  # covers the elu1 drain
            proj_layer(w2_sb, elu1, l2_emit(h1_sb))
            normalize(h1_sb, n1_f8, 0)
            nc.scalar.dma_start(ag1_in[:], n1_f8[:])
            nc.gpsimd.collective_compute(
                "AllGather", ALU.bypass, replica_groups=rg,
                ins=[ag1_in[:].opt()], outs=[ag1_out[:].opt()],
            )
            proj_layer(w1_sb, z2_sb, l1e2, ots=range(2, KO))
            proj_layer(w2_sb, elu2, l2_emit(h2_sb))
            normalize(h2_sb, n2_f8, 1)
            nc.scalar.dma_start(ag2_in[:], n2_f8[:])
            nc.gpsimd.collective_compute(
                "AllGather", ALU.bypass, replica_groups=rg,
                ins=[ag2_in[:].opt()], outs=[ag2_out[:].opt()],
            )

            # ------ p_i = n1_i . n2_i via bf16 h1*h2 and f32 1/norms ------
            pps = [
                psB.tile([16, 512], F32, name=f"pps{c}", tag="ps_wide") for c in range(2)
            ]
            for kt in range(KO):
                hq = scratch.tile([P, BLK], BF, tag="sq", name=f"hq{kt}")
                nc.vector.tensor_tensor(hq[:], h1_sb[:, kt, :], h2_sb[:, kt, :], ALU.mult)
                for ch in range(2):
                    nc.tensor.matmul(
                        pps[ch][0:1, :],
                        ones_bf[:],
                        hq[:, bass.ts(ch, 512)],
                        start=(kt == 0),
                        stop=(kt == KO - 1),
                    )
            for ch in range(2):
                sl = bass.ts(ch, 512)
                p_c = small.tile([1, 512], F32, tag="ssq_c", name=f"p_c{ch}")
                nc.vector.tensor_copy(p_c[:], pps[ch][0:1, :])
                nc.vector.tensor_tensor(p_c[:], p_c[:], rn_f[0][:, sl], ALU.mult)
                nc.vector.tensor_tensor(p_c[:], p_c[:], rn_f[1][:, sl], ALU.mult)
                nc.gpsimd.dma_start(p_dram[ch * 512 : (ch + 1) * 512], p_c[:])

            # ---------------- similarity passes ----------------
            def rhs_load(ag, j, nm):
                t = rhsp.tile([P, KO, BLK], F8, tag="rhs", name=nm)
                blk = ag[j]
                nc.sync.dma_start(t[:, 0:4, :], blk[:, 0:4, :])
                nc.sync.dma_start(t[:, 4:8, :], blk[:, 4:8, :])
                return t

            # colsums deferred to the next strip-group so the PE never waits
            # on the Act engine's exp outputs
            pending = []

            def flush_pending():
                while pending:
                    pending.pop(0)()

            def colsum(j, rs_slot, h, ex_tiles, nm):
                """PSUM-accumulated fp8 ones DoubleRow colsum -> rs_in[j, slot, h]."""

                def emit():
                    cps = psB.tile([16, 512], F32, tag="ps_wide", name=f"cps{nm}")
                    for i, (ext, esl) in enumerate(ex_tiles):
                        nc.tensor.matmul(
                            cps[:],
                            ones8[:],
                            ext[:, :, esl] if esl is not None else ext[:],
                            start=(i == 0),
                            stop=(i == len(ex_tiles) - 1),
                            perf_mode=DR,
                        )
                    cst = scratch.tile([1, 512], F32, tag="cst", bufs=2, name=f"cst{nm}")
                    nc.vector.tensor_copy(cst[:], cps[0:1, :])
                    if rs_slot == 2:
                        dst = rs_in_b[j : j + 1, bass.ds(h * 512, 512)]
                    else:
                        dst = rs_in_a[
                            j : j + 1, rs_slot : rs_slot + 1, bass.ds(h * 512, 512)
                        ]
                    nc.gpsimd.dma_start(dst, cst[:])

                pending.append(emit)

            def sym_pair(lhs, ag, j0, Dp, rs_slot, nm):
                """Two symmetric-half strips (j0, j0+1), lhs shared per matmul
                pair: Q01 full + Q00/Q11 at exp weight 1/2."""
                rt0 = rhs_load(ag, j0, f"r{nm}a")
                rt1 = rhs_load(ag, j0 + 1, f"r{nm}b")
                ex = {}  # (strip, quad, tt-pair) -> fp8 exp tile
                first = True
                # Q00 tts 0-3 cols 0:512 w=1/2; Q01 tts 0-3 cols 512: w=1;
                # Q11 tts 4-7 cols 512: w=1/2
                for quad, tts, col, half in (
                    ("q00", range(4), 0, True),
                    ("q01", range(4), 512, False),
                    ("q11", range(4, 8), 512, True),
                ):
                    for tt in tts:
                        ps = psA.tile([P, 1024], F32, tag="ps_big", name=f"p{nm}{quad}{tt}")
                        dr_multi(
                            [ps[:, 0:512], ps[:, 512:1024]],
                            lhs, tt, [(rt0, col), (rt1, col)],
                        )
                        if first:
                            flush_pending()
                            first = False
                        for s in range(2):
                            slot = 2 * (j0 + s) + (0 if quad == "q01" else 1)
                            key = (s, quad, tt // 2)
                            if key not in ex:
                                ex[key] = expp.tile(
                                    [P, 2, 512], F8, tag="exq", bufs=24,
                                    name=f"x{nm}{s}{quad}{tt // 2}",
                                )
                            nc.scalar.activation(
                                ex[key][:, tt % 2, :], ps[:, bass.ts(s, 512)], AF.Exp,
                                scale=SC, bias=(lnhalf[:] if half else 0.0),
                                accum_out=Dp[:, tt, slot : slot + 1],
                            )
                for s in range(2):
                    colsum(
                        j0 + s, rs_slot, 0,
                        [(ex[(s, "q00", 0)], None), (ex[(s, "q00", 1)], None)],
                        f"a{nm}{s}",
                    )
                    colsum(
                        j0 + s, rs_slot, 1,
                        [(ex[(s, "q01", 0)], None), (ex[(s, "q01", 1)], None),
                         (ex[(s, "q11", 2)], None), (ex[(s, "q11", 3)], None)],
                        f"b{nm}{s}",
                    )

            # S11 (needs AG1 only; overlaps AG2)
            for j0 in range(0, NCORES, 2):
                sym_pair(n1_f8, ag1_out, j0, D1p, 0, f"s11_{j0}")
            # S22 (needs AG2)
            for j0 in range(0, NCORES, 2):
                sym_pair(n2_f8, ag2_out, j0, D2p, 1, f"s22_{j0}")
            # S12 full strips (lhs n1, rhs gathered n2)
            for j in range(NCORES):
                rt = rhs_load(ag2_out, j, f"s12_{j}")
                if j == 1:
                    # S11/S22 colsums flushed during j==0 -> reduce them now,
                    # overlapping the remaining S12 strips
                    nc.gpsimd.collective_compute(
                        "ReduceScatter", ALU.add, replica_groups=rg,
                        ins=[rs_in_a[:].opt()], outs=[rs_out_a[:].opt()],
                    )
                exf = [
                    expp.tile([P, 2, 1024], F8, tag="exf", bufs=8, name=f"xf{j}_{i}")
                    for i in range(4)
                ]
                for tt in range(NT):
                    ps = psA.tile([P, 1024], F32, tag="ps_big", name=f"pf{j}_{tt}")
                    dr_multi(
                        [ps[:, 0:512], ps[:, 512:1024]],
                        n1_f8, tt, [(rt, 0), (rt, 512)],
                    )
                    if tt == 0:
                        flush_pending()
                    nc.scalar.activation(
                        exf[tt // 2][:, tt % 2, :], ps[:], AF.Exp, scale=SC,
                        accum_out=D1p[:, tt, 2 * NCORES + j : 2 * NCORES + j + 1],
                    )
                for h in range(2):
                    colsum(
                        j, 2, h,
                        [(t, bass.ds(h * 512, 512)) for t in exf],
                        f"f{j}_{h}",
                    )
            flush_pending()

            nc.gpsimd.collective_compute(
                "ReduceScatter", ALU.add, replica_groups=rg,
                ins=[rs_in_b[:].opt()], outs=[rs_out_b[:].opt()],
            )

            # ---------------- final loss ----------------
            # pm = -2p is ready long before the sims end
            p2 = small.tile([P, NT], F32, tag="p2")
            nc.sync.dma_start(p2[:], pt(p_dram[:]))
            pm = small.tile([P, NT], F32, tag="pm")
            nc.vector.tensor_scalar(pm[:], p2[:], -2.0, None, ALU.mult)

            r1s = small.tile([P, NT], F32, tag="r1s")
            r2s = small.tile([P, NT], F32, tag="r2s")
            nc.vector.reduce_sum(r1s[:], D1p[:], axis=mybir.AxisListType.X)
            nc.vector.reduce_sum(r2s[:], D2p[:], axis=mybir.AxisListType.X)
            c1 = small.tile([P, NT], F32, tag="c1")
            c2a = small.tile([P, NT], F32, tag="c2a")
            c2b = small.tile([P, NT], F32, tag="c2b")
            nc.sync.dma_start(c1[:], pt(rs_out_a[0]))
            nc.sync.dma_start(c2a[:], pt(rs_out_a[1]))
            nc.sync.dma_start(c2b[:], pt(rs_out_b[:]))

            d1 = small.tile([P, NT], F32, tag="d1")
            d2 = small.tile([P, NT], F32, tag="d2")
            nc.vector.tensor_tensor(d1[:], r1s[:], c1[:], ALU.add)
            nc.vector.tensor_tensor(d2[:], r2s[:], c2a[:], ALU.add)
            nc.vector.tensor_tensor(d2[:], d2[:], c2b[:], ALU.add)
            l1 = small.tile([P, NT], F32, tag="l1")
            l2 = small.tile([P, NT], F32, tag="l2")
            nc.scalar.activation(l1[:], d1[:], AF.Ln, bias=negE2[:])
            nc.scalar.activation(l2[:], d2[:], AF.Ln, bias=negE2[:])
            loss = small.tile([P, NT], F32, tag="loss")
            nc.vector.tensor_tensor(loss[:], l1[:], l2[:], ALU.add)
            nc.vector.tensor_scalar(loss[:], loss[:], 0.5, None, ALU.mult)
            nc.vector.tensor_tensor(loss[:], loss[:], pm[:], ALU.add)
            nc.sync.dma_start(pt(out[:]), loss[:])

    nc.finalize()
    return nc


@lru_cache(maxsize=1)
def _built():
    return _build()


def _kimajor(a):
    """[D, X] -> [ki=128, ko=8, X] contiguous device layout."""
    d, x = a.shape
    return np.ascontiguousarray(a.reshape(KO, P, x).transpose(1, 0, 2))


def _prep_inputs(z1, z2, fc1_w, fc1_b, fc2_w, fc2_b):
    f8 = ml_dtypes.float8_e4m3
    w1 = np.asarray(fc1_w, np.float32)
    w2 = np.asarray(fc2_w, np.float32)
    w1t = _kimajor(np.ascontiguousarray(w1.T).astype(f8))
    w2t = _kimajor(np.ascontiguousarray(w2.T).astype(f8))
    b1 = np.asarray(fc1_b, np.float32)
    # device computes (elu+1) @ W2.T; correct with the fp8-rounded W2 row sums
    b2p = (
        np.asarray(fc2_b, np.float32) - w2.astype(f8).astype(np.float32).sum(axis=1)
    ).astype(np.float32)
    in_maps = []
    for c in range(NCORES):
        sl = slice(c * BLK, (c + 1) * BLK)
        in_maps.append(
            {
                "z1t": _kimajor(np.asarray(z1[sl], np.float32).T.astype(f8)),
                "z2t": _kimajor(np.asarray(z2[sl], np.float32).T.astype(f8)),
                "w1t": w1t,
                "w2t": w2t,
                "b1": b1,
                "b2p": b2p,
            }
        )
    return in_maps


def _install_ntff_shim():
    """Register the axon NTFF profile hook (antenv.axon_hooks is absent in
    this image; rebuild it from trn_agent_boot's ctypes recipe)."""
    import sys
    import types

    if "antenv.axon_hooks" in sys.modules:
        return True
    try:
        import antenv
        from trn_agent_boot.trn_boot import _ntff_profile_via_ctypes

        hook = _ntff_profile_via_ctypes("/opt/axon/libaxon_pjrt.so")
        if hook is None:
            return False
        m = types.ModuleType("antenv.axon_hooks")
        m._hook = hook
        m.get_axon_ntff_profile_hook = lambda: m._hook
        m.set_axon_ntff_profile_hook = lambda h: setattr(m, "_hook", h)
        sys.modules["antenv.axon_hooks"] = m
        antenv.axon_hooks = m
        # artifact upload needs egress; neuter it for local profiling
        import concourse.bass_utils as _bu

        _bu.upload_artifacts = lambda tmpdir: f"file://{tmpdir}"
        return True
    except Exception as e:
        print(f"ntff shim unavailable: {e!r}")
        return False


def _run(in_maps, trace=False):
    nc = _built()
    if trace and not _install_ntff_shim():
        trace = False
    last = None
    for attempt in range(3):
        try:
            res = run_bass_kernel_spmd(nc, in_maps, list(range(NCORES)), trace=trace)
            if all(np.isfinite(res.results[c]["out"]).all() for c in range(NCORES)):
                return res
            print("nonfinite output, retrying")
        except Exception as e:  # device occasionally wedged from a prior process
            last = e
            if "UNRECOVERABLE" not in str(e) and "UNAVAILABLE" not in str(e):
                raise
            print(f"device error (attempt {attempt}): retrying")
    if last is not None:
        raise last
    return res


def kernel(z1, z2, fc1_w, fc1_b, fc2_w, fc2_b):
    in_maps = _prep_inputs(z1, z2, fc1_w, fc1_b, fc2_w, fc2_b)
    res = _run(in_maps, trace=os.environ.get("KERNEL_TRACE", "") == "1")
    if res.exec_time_ns is not None:
        print(f"HW exec time: {res.exec_time_ns} ns")
    out = np.concatenate([res.results[c]["out"] for c in range(NCORES)])
    return out.astype(np.float32)
